# revision 1
# baseline (speedup 1.0000x reference)
"""LQR step (Riccati backward recursion + forward rollout) on 8 Trainium2 NeuronCores.

Sharding: pure data-parallel over the batch dimension B=256 -> 8 shards of 32
(one per NeuronCore), per the problem's sharding hint. The T=50 recursion is
sequential but every timestep op is batch-parallel, so there is no cross-device
communication inside the Riccati loop.

The per-batch 32x32 SPD solve (Q_uu) is done with a Newton-Schulz iteration
(pure batched matmuls, quadratically convergent; Q_uu >= I by construction and
a per-batch Gershgorin bound gives the scaling), which maps onto the tensor
engine far better than a pivoted LU.
"""

import numpy as np
import jax
import jax.numpy as jnp
from jax import lax

N_STATE, N_CTRL, T, B = 64, 32, 50, 256
N = N_STATE + N_CTRL
M_DEV = 8
B_LOC = B // M_DEV
NS_ITERS = 10

_cache = {}


def _ns_inv(A):
    # A: [b, n, n] SPD with lambda_min >= 1. Newton-Schulz inverse.
    L = jnp.max(jnp.sum(jnp.abs(A), axis=-1), axis=-1)          # Gershgorin upper bound
    alpha = 2.0 / (1.0 + L)
    eye = jnp.eye(A.shape[-1], dtype=A.dtype)
    X = alpha[:, None, None] * eye
    I2 = 2.0 * eye
    for _ in range(NS_ITERS):
        X = X @ (I2 - A @ X)
    return X


def _lqr_shard(x_init, C, c, F, current_x, current_u):
    ns = N_STATE
    xut = jnp.concatenate([current_x, current_u], axis=-1)       # [T,b,N]
    c_back = jnp.einsum('tbij,tbj->tbi', C, xut) + c             # [T,b,N]

    def bwd_step(carry, inp):
        V, v = carry
        Ct, ct, Ft = inp
        FtT = jnp.swapaxes(Ft, -1, -2)
        W = V @ Ft                                               # [b,ns,N]
        Qt = Ct + FtT @ W
        qt = ct + jnp.einsum('bij,bi->bj', Ft, v)
        Q_xx = Qt[:, :ns, :ns]
        Q_xu = Qt[:, :ns, ns:]
        Q_ux = Qt[:, ns:, :ns]
        Q_uu = Qt[:, ns:, ns:]
        q_x = qt[:, :ns]
        q_u = qt[:, ns:]
        Quu_inv = _ns_inv(Q_uu)
        Kt = -(Quu_inv @ Q_ux)                                   # [b,nc,ns]
        kt = -jnp.einsum('bij,bj->bi', Quu_inv, q_u)             # [b,nc]
        # Schur-complement forms (exact simplifications of the reference
        # expressions given Kt = -Quu^-1 Q_ux, kt = -Quu^-1 q_u, Qt symmetric):
        Vn = Q_xx + Q_xu @ Kt
        vn = q_x + jnp.einsum('bij,bj->bi', Q_xu, kt)
        Vn = 0.5 * (Vn + jnp.swapaxes(Vn, -1, -2))
        return (Vn, vn), (Kt, kt)

    bsz = x_init.shape[0]
    V0 = jnp.zeros((bsz, ns, ns), C.dtype)
    v0 = jnp.zeros((bsz, ns), C.dtype)
    _, (K_rev, k_rev) = lax.scan(bwd_step, (V0, v0),
                                 (C[::-1], c_back[::-1], F[::-1]))
    K = K_rev[::-1]
    k = k_rev[::-1]

    x_ref_next = jnp.concatenate([current_x[1:], jnp.zeros_like(current_x[:1])], 0)

    def fwd_step(carry, inp):
        xt, dxt = carry
        Kt, kt, ut, Ft, xr = inp
        new_ut = jnp.einsum('bij,bj->bi', Kt, dxt) + ut + kt
        new_xut = jnp.concatenate([xt, new_ut], axis=-1)
        x_next = jnp.einsum('bij,bj->bi', Ft, new_xut)
        return (x_next, x_next - xr), (xt, new_ut)

    _, (new_x, new_u) = lax.scan(fwd_step, (x_init, jnp.zeros_like(x_init)),
                                 (K, k, current_u, F, x_ref_next))
    return new_x, new_u


def _get_fn():
    if 'fn' not in _cache:
        devs = [d for d in jax.devices() if d.platform != 'cpu'][:M_DEV]
        if len(devs) < M_DEV:
            devs = jax.devices()[:M_DEV]
        _cache['fn'] = jax.pmap(_lqr_shard, devices=devs)
    return _cache['fn']


def _shard(a):
    # [..., B, ...] with batch at axis 1 for T-leading tensors, axis 0 for x_init
    if a.shape[0] == T:
        # [T, B, ...] -> [M, T, B_LOC, ...]
        return np.moveaxis(a.reshape(T, M_DEV, B_LOC, *a.shape[2:]), 1, 0)
    # [B, ...] -> [M, B_LOC, ...]
    return a.reshape(M_DEV, B_LOC, *a.shape[1:])


def kernel(x_init, C, c, F, current_x, current_u):
    fn = _get_fn()
    args = [np.ascontiguousarray(_shard(np.asarray(a, np.float32)))
            for a in (x_init, C, c, F, current_x, current_u)]
    new_x, new_u = fn(*args)
    new_x = np.asarray(new_x)   # [M, T, B_LOC, ns]
    new_u = np.asarray(new_u)
    # [M, T, B_LOC, d] -> [T, M, B_LOC, d] -> [T, B, d]
    new_x = np.moveaxis(new_x, 0, 1).reshape(T, B, N_STATE)
    new_u = np.moveaxis(new_u, 0, 1).reshape(T, B, N_CTRL)
    return new_x, new_u


if __name__ == '__main__':
    rng = np.random.default_rng(0)
    demo = {
        'x_init': rng.standard_normal((B, N_STATE), dtype=np.float32),
        'C': rng.standard_normal((T, B, N, N), dtype=np.float32) * 0.01,
        'c': rng.standard_normal((T, B, N), dtype=np.float32),
        'F': rng.standard_normal((T, B, N_STATE, N), dtype=np.float32) * 0.05,
        'current_x': rng.standard_normal((T, B, N_STATE), dtype=np.float32),
        'current_u': rng.standard_normal((T, B, N_CTRL), dtype=np.float32),
    }
    demo['C'] = np.einsum('tbij,tbkj->tbik', demo['C'], demo['C']) + np.eye(N, dtype=np.float32)
    out = kernel(**demo)
    print([o.shape for o in out])
